# revision 22
# baseline (speedup 1.0000x reference)
"""Trainium2 Bass kernel for causal multi-head attention (B=2, T=4096, C=768, H=12).

Sharding: 8 cores = 2 (batch) x 4 (head groups of 3). Each core computes, for its
batch element b and its 3 heads:
  - Q^T/K^T projection (transposed layout, fused q/k bias)
  - V projection (natural layout, ones column appended for free softmax sums)
  - causal flash attention in S^T = [keys, queries] orientation
  - row-sharded output projection -> partial [T, C] output
Host sums the 4 partial outputs per batch element and adds the bias terms.

All matmuls run as float32r (FP22: 1 row/cycle at N>=256), fp32 accumulate.

v4 structure (from trace analysis of earlier iterations):
  - Q^T and K^T live in SEPARATE SBUF tiles (same-tile operands serialize the PE).
  - Flash attention is software-pipelined at single-key-chunk granularity with a
    depth-4 pending queue: the S matmul of item i+4 issues before exp/mask/AV of
    item i, so the PE never waits on the ACT engine's exp or the DVE mask.
  - The NEXT block's QKV projection is emitted before this block's output
    projection, hiding the softmax-normalize tail under projection matmuls.
  - Output projection contracts heads 0+1 jointly at K=128 (y0 at partitions
    0-63, y1 DMA-shifted to 64-127), plus one K=64 matmul for head 2.
  - Softmax denominators: ACT Copy (in every table set -> no table reload) stages
    the psum sums row; reciprocal_approx_fast on DVE; partition broadcast on
    GPSIMD.
  - Diagonal-chunk trims are clamped to keep f32r matmul moving dims >= 256
    (below 256 the PE drops to 1/4 rate for f32r).
"""

import os
import sys

for _p in ("/opt/trn_rl_repo", "/root/.axon_site/_ro/trn_rl_repo"):
    if os.path.isdir(_p) and _p not in sys.path:
        sys.path.insert(0, _p)

import ml_dtypes
import numpy as np

import concourse.bass as bass
import concourse.mybir as mybir
import concourse.tile as tile
from concourse import bacc, bass_utils

# Problem constants (hardcoded per harness contract)
B, T_FULL, C = 2, 4096, 768
H, D = 12, 64
N_CORES = 8
HPC = 3          # heads per core
GROUPS = 4       # head groups (cores per batch element)

F32 = mybir.dt.float32
F32R = mybir.dt.float32r


def build_nc(T=T_FULL):
    """Build the per-core Bass module. T must be a multiple of 512."""
    QB = 512                 # query block
    KC = 128                 # key chunk
    NTB = T // QB            # token blocks
    NCC = C // 128           # contraction chunks (6)
    NKC = T // KC            # key chunks total
    CS = HPC * (D + 1)       # V|ones chunk stride (195)
    CO = C                   # output channels
    VW = HPC * D             # v width (192)
    VWP = 256                # padded v proj width

    nc = bacc.Bacc(None, target_bir_lowering=False, debug=False)

    xT_d = nc.dram_tensor("xT", [C, T], F32R, kind="ExternalInput")
    wqk_d = nc.dram_tensor("wqk", [C, 4 * 128], F32R, kind="ExternalInput")
    bqk_d = nc.dram_tensor("bqk", [128, 4], F32, kind="ExternalInput")
    wv_d = nc.dram_tensor("wv", [C, VWP], F32R, kind="ExternalInput")
    wout_d = nc.dram_tensor("wout", [VW, CO], F32R, kind="ExternalInput")
    masks_d = nc.dram_tensor("masks", [128, 4 * QB], F32R, kind="ExternalInput")
    ident_d = nc.dram_tensor("ident", [128, 64], F32R, kind="ExternalInput")
    out_d = nc.dram_tensor("out", [T, CO], F32, kind="ExternalOutput")

    # per-head addressing into qt/kt tiles: block 0 = h0@p0-63|h1@p64-127,
    # block 1 = h2@p0-63
    def qbase(h):
        return 64 if h == 1 else 0

    def hoff(h):
        return T if h == 2 else 0

    with tile.TileContext(nc) as tc:
        with (
            tc.tile_pool(name="singles", bufs=1) as singles,
            tc.tile_pool(name="xt", bufs=12) as xt_pool,
            tc.tile_pool(name="e", bufs=3) as e_pool,
            tc.tile_pool(name="yt", bufs=4) as yt_pool,
            tc.tile_pool(name="nrm", bufs=2) as nrm_pool,
            tc.tile_pool(name="ostage", bufs=2) as out_pool,
            tc.tile_pool(name="ps", bufs=3, space="PSUM") as psum_s,
            tc.tile_pool(name="pz", bufs=2, space="PSUM") as psum_z,
        ):
            # Persistent SBUF tensors
            qt = singles.tile([128, 2 * T], F32R)
            # K^T per head, zero-padded to 128 contraction rows: the zero
            # half annihilates the other head's Q rows in the shared qt rhs,
            # letting every S matmul run at K=128 (matching AV's shape --
            # mixed-K interleave costs ~50ns/matmul extra in weight loads).
            kt0 = singles.tile([128, T], F32R)
            kt1 = singles.tile([128, T], F32R)
            kt2 = singles.tile([128, T], F32R)
            v1 = singles.tile([128, NKC * CS], F32R)      # V|ones, keys on partitions
            wqk_s = singles.tile([128, NCC * 512], F32R)
            wv_s = singles.tile([128, NCC * VWP], F32R)
            wout_s = singles.tile([64, HPC * CO], F32R)
            masks_s = singles.tile([128, 4 * QB], F32R)
            bqk_s = singles.tile([128, 4], F32)
            ident_s = singles.tile([128, 64], F32R)

            # ones columns of v1 (memset can't write f32r; DVE copy rounds)
            ones_c = singles.tile([128, 1], F32)
            nc.vector.memset(ones_c[:], 1.0)
            ones_dst = v1[:].rearrange("p (k h x) -> p k h x", h=HPC, x=D + 1)[
                :, :, :, D:D + 1
            ]
            nc.vector.tensor_copy(ones_dst, ones_c.to_broadcast([128, NKC, HPC, 1]))
            zero_c = singles.tile([64, 1], F32)
            nc.vector.memset(zero_c[:], 0.0)
            nc.vector.tensor_copy(kt0[64:128, :], zero_c.to_broadcast([64, T]))
            nc.vector.tensor_copy(kt1[0:64, :], zero_c.to_broadcast([64, T]))
            nc.vector.tensor_copy(kt2[64:128, :], zero_c.to_broadcast([64, T]))
            # qt block 1 upper half is never written but is read (and
            # annihilated by kt2's zero rows) by h2's K=128 S matmuls --
            # zero it so stray NaN bit patterns can't poison 0*x.
            nc.vector.tensor_copy(qt[64:128, T:2 * T], zero_c.to_broadcast([64, T]))
            nc.sync.dma_start(out=bqk_s[:], in_=bqk_d.ap())
            nc.sync.dma_start(out=ident_s[:], in_=ident_d.ap())

            def issue_xt_dma(tb):
                lst = []
                for c in range(NCC):
                    t_ = xt_pool.tile([128, QB], F32R, tag="xt", name=f"xt{tb}_{c}")
                    nc.sync.dma_start(
                        out=t_[:],
                        in_=xT_d.ap()[c * 128:(c + 1) * 128, tb * QB:(tb + 1) * QB],
                    )
                    lst.append(t_)
                return lst

            # ---- skewed cross-block pipeline ----
            # One global pending queue of attention items spans all blocks.
            # All transient psum (S scores, qk/v projections, outproj) shares
            # the 3-buffer [128,1024] "ps" ring; before each ring allocation,
            # pending items whose score tile is about to be recycled are
            # drained (exp/mask/AV emitted). This lets the next block's h0
            # attention items interleave with projections and outproj, so the
            # ACT engine always has exp work queued while the PE streams
            # projection matmuls.
            kts = [kt0, kt1, kt2]
            pending = []
            pzs = {}
            ps_ctr = [0]

            def emit_eav(j, h, m, ps, yts):
                nb = 2 * (j + 1)
                pz = pzs[(j, h)]
                e = e_pool.tile([128, 2 * QB], F32R, tag="e", name=f"e{j}_{h}_{m}")
                trims = [max(0, (2 * m + u - 4 * j) * KC) for u in range(2)]
                if trims[0] == 0 and trims[1] == 0:
                    nc.scalar.activation(
                        e[:], ps[:], mybir.ActivationFunctionType.Exp,
                        scale=0.125,
                    )
                else:
                    for u in range(2):
                        lo = u * QB + trims[u]
                        nc.scalar.activation(
                            e[:, lo:(u + 1) * QB], ps[:, lo:(u + 1) * QB],
                            mybir.ActivationFunctionType.Exp,
                            scale=0.125,
                        )
                for u in range(2):
                    cdiag = 2 * m + u - 4 * j
                    if cdiag >= 0:
                        trim = trims[u]
                        nc.vector.tensor_mul(
                            e[:, u * QB + trim:(u + 1) * QB],
                            e[:, u * QB + trim:(u + 1) * QB],
                            masks_s[:, cdiag * QB + trim:(cdiag + 1) * QB],
                        )
                for u in range(2):
                    n = 2 * m + u
                    trim = trims[u]
                    nc.tensor.matmul(
                        pz[0:D + 1, trim:QB],
                        lhsT=(v1[:, n * CS + h * (D + 1): n * CS + (h + 1) * (D + 1)]),
                        rhs=(e[:, u * QB + trim:(u + 1) * QB]),
                        start=(m == 0 and u == 0),
                        stop=(m == nb - 1 and u == 1),
                    )
                if m == nb - 1:
                    # normalize: y = z * (1/sums). ACT Copy stages the sums row
                    # from psum partition 64 to sbuf partition 0 (Copy is in
                    # every ACT table set -> no table reload); then DVE
                    # reciprocal + GPSIMD partition broadcast.
                    sums = nrm_pool.tile([1, QB], F32, tag="sums")
                    nc.scalar.activation(
                        sums[:], pz[D:D + 1, 0:QB],
                        mybir.ActivationFunctionType.Copy,
                    )
                    rc = nrm_pool.tile([1, QB], F32, tag="rc")
                    nc.vector.reciprocal_approx_fast(out=rc[:], in_=sums[:])
                    bc = nrm_pool.tile([64, QB], F32, tag="bc")
                    nc.gpsimd.partition_broadcast(bc[:], rc[:])
                    yt = yt_pool.tile([64, QB], F32R, tag="yt", name=f"yt{j}_{h}")
                    nc.vector.tensor_mul(yt[:], pz[0:D, 0:QB], bc[:])
                    yts.append(yt)
                    del pzs[(j, h)]

            def alloc_ps(name):
                # ps ring has 3 buffers: alloc #c reuses the buffer of alloc
                # #(c-3), so any pending item holding that tile must have its
                # consumers (exp) emitted before this allocation.
                c = ps_ctr[0]
                while pending and pending[0][5] <= c - 3:
                    j_, h_, m_, ps_, yts_, _ = pending.pop(0)
                    emit_eav(j_, h_, m_, ps_, yts_)
                t = psum_s.tile([128, 2 * QB], F32, tag="ps", name=name)
                ps_ctr[0] += 1
                return t

            def push(j, h, m, yts):
                if m == 0:
                    pzs[(j, h)] = psum_z.tile(
                        [128, 512], F32, tag="pz", name=f"pz{j}_{h}"
                    )
                ps = alloc_ps(f"s{j}_{h}_{m}")
                ho_ = hoff(h)
                for u in range(2):
                    n = 2 * m + u
                    trim = max(0, (n - 4 * j) * KC)
                    nc.tensor.matmul(
                        ps[:, u * QB + trim:(u + 1) * QB],
                        lhsT=(kts[h][:, n * KC:(n + 1) * KC]),
                        rhs=(qt[:, ho_ + j * QB + trim: ho_ + (j + 1) * QB]),
                        start=True,
                        stop=True,
                    )
                pending.append((j, h, m, ps, yts, ps_ctr[0] - 1))

            def flush():
                while pending:
                    j_, h_, m_, ps_, yts_, _ = pending.pop(0)
                    emit_eav(j_, h_, m_, ps_, yts_)

            def emit_qkproj_mt(tb, xt, mt):
                # M-tiles: 0 = Q(h0)|Q(h1), 1 = K(h0)|K(h1), 2 = Q(h2)|K(h2).
                # K(h2) lands on psum partitions 64-127 but must live at 0-63:
                # identity-shift via PE.
                ps = alloc_ps(f"pj{tb}_{mt}")
                for c in range(NCC):
                    nc.tensor.matmul(
                        ps[:, 0:QB],
                        lhsT=(wqk_s[:, c * 512 + mt * 128: c * 512 + (mt + 1) * 128]),
                        rhs=(xt[c][:]),
                        start=(c == 0),
                        stop=(c == NCC - 1),
                    )
                if mt == 0:
                    nc.vector.tensor_scalar(
                        out=qt[:, tb * QB:(tb + 1) * QB],
                        in0=ps[:, 0:QB],
                        scalar1=bqk_s[:, 0:1],
                        scalar2=None,
                        op0=mybir.AluOpType.add,
                    )
                elif mt == 1:
                    nc.vector.tensor_scalar(
                        out=kt0[0:64, tb * QB:(tb + 1) * QB],
                        in0=ps[0:64, 0:QB],
                        scalar1=bqk_s[0:64, 1:2],
                        scalar2=None,
                        op0=mybir.AluOpType.add,
                    )
                    nc.vector.tensor_scalar(
                        out=kt1[64:128, tb * QB:(tb + 1) * QB],
                        in0=ps[64:128, 0:QB],
                        scalar1=bqk_s[64:128, 1:2],
                        scalar2=None,
                        op0=mybir.AluOpType.add,
                    )
                else:
                    nc.vector.tensor_scalar(
                        out=qt[0:64, T + tb * QB: T + (tb + 1) * QB],
                        in0=ps[0:64, 0:QB],
                        scalar1=bqk_s[0:64, 2:3],
                        scalar2=None,
                        op0=mybir.AluOpType.add,
                    )
                    ktmp = e_pool.tile([128, 2 * QB], F32R, tag="e", name=f"ktmp{tb}")
                    nc.vector.tensor_scalar(
                        out=ktmp[64:128, 0:QB],
                        in0=ps[64:128, 0:QB],
                        scalar1=bqk_s[64:128, 2:3],
                        scalar2=None,
                        op0=mybir.AluOpType.add,
                    )
                    ps2 = alloc_ps(f"pk{tb}")
                    nc.tensor.matmul(
                        ps2[0:64, 0:QB],
                        lhsT=(ident_s[64:128, :]),
                        rhs=(ktmp[64:128, 0:QB]),
                        start=True,
                        stop=True,
                    )
                    nc.vector.tensor_copy(
                        kt2[0:64, tb * QB:(tb + 1) * QB],
                        ps2[0:64, 0:QB],
                    )

            def emit_vproj_ts(tb, xt, ts):
                pv = alloc_ps(f"pv{tb}_{ts}")
                for c in range(NCC):
                    nc.tensor.matmul(
                        pv[:, 0:VWP],
                        lhsT=(xt[c][:, ts * 128:(ts + 1) * 128]),
                        rhs=(wv_s[:, c * VWP:(c + 1) * VWP]),
                        start=(c == 0),
                        stop=(c == NCC - 1),
                    )
                kc = tb * (QB // 128) + ts
                dst = v1[:, kc * CS:(kc + 1) * CS].rearrange(
                    "p (h x) -> p h x", x=D + 1
                )[:, :, 0:D]
                src = pv[:, 0:VW].rearrange("p (h d) -> p h d", d=D)
                nc.vector.tensor_copy(dst, src)

            def emit_outproj_ts(tb, yts, ts):
                ot = out_pool.tile([128, CO], F32, tag="ot", name=f"ot{tb}_{ts}")
                for half in range(2):
                    po = alloc_ps(f"po{tb}_{ts}_{half}")
                    for h in range(HPC):
                        nc.tensor.matmul(
                            po[:, 0:384],
                            lhsT=(yts[h][:, ts * 128:(ts + 1) * 128]),
                            rhs=(wout_s[:, h * CO + half * 384: h * CO + (half + 1) * 384]),
                            start=(h == 0),
                            stop=(h == HPC - 1),
                        )
                    nc.vector.tensor_copy(
                        ot[:, half * 384:(half + 1) * 384], po[:, 0:384]
                    )
                row = tb * QB + ts * 128
                nc.sync.dma_start(out=out_d.ap()[row:row + 128, :], in_=ot[:])

            # ---- main schedule ----
            ydict = {}
            xt_cur = issue_xt_dma(0)
            for c in range(NCC):
                nc.sync.dma_start(
                    out=wqk_s[:, c * 512:(c + 1) * 512],
                    in_=wqk_d.ap()[c * 128:(c + 1) * 128, :],
                )
                nc.sync.dma_start(
                    out=wv_s[:, c * VWP:(c + 1) * VWP],
                    in_=wv_d.ap()[c * 128:(c + 1) * 128, :],
                )
            nc.sync.dma_start(out=masks_s[:], in_=masks_d.ap())
            for h_ in range(HPC):
                nc.sync.dma_start(
                    out=wout_s[:, h_ * CO:(h_ + 1) * CO],
                    in_=wout_d.ap()[h_ * D:(h_ + 1) * D, :],
                )
            for mt in range(3):
                emit_qkproj_mt(0, xt_cur, mt)
            for ts in range(QB // 128):
                emit_vproj_ts(0, xt_cur, ts)
            ydict[0] = []
            for m in range(2):
                push(0, 0, m, ydict[0])

            for tb in range(NTB):
                j = tb
                nb = 2 * (j + 1)
                yts = ydict[tb]
                xt_next = issue_xt_dma(tb + 1) if tb + 1 < NTB else None
                for m in range(nb):
                    push(j, 1, m, yts)
                for m in range(nb):
                    push(j, 2, m, yts)
                if tb + 1 < NTB:
                    jn = tb + 1
                    ydict[jn] = []
                    emit_qkproj_mt(jn, xt_next, 0)
                    emit_qkproj_mt(jn, xt_next, 1)
                    work = [lambda jn=jn, x=xt_next: emit_qkproj_mt(jn, x, 2)]
                    work += [
                        (lambda jn=jn, x=xt_next, t=t: emit_vproj_ts(jn, x, t))
                        for t in range(QB // 128)
                    ]
                    work += [
                        (lambda tb=tb, y=yts, t=t: emit_outproj_ts(tb, y, t))
                        for t in range(QB // 128)
                    ]
                    pushes = [
                        (lambda jn=jn, m=m: push(jn, 0, m, ydict[jn]))
                        for m in range(2 * (jn + 1))
                    ]
                    pi = wi = 0
                    while pi < len(pushes) or wi < len(work):
                        if pi < len(pushes):
                            pushes[pi]()
                            pi += 1
                        if wi < len(work):
                            work[wi]()
                            wi += 1
                    ydict.pop(tb)
                else:
                    flush()
                    for ts in range(QB // 128):
                        emit_outproj_ts(tb, yts, ts)
                    ydict.pop(tb)

    nc.compile()
    return nc


def make_masks():
    """Diagonal-block masks: masks[k, c*512 + q] = 1.0 iff 128*c + k <= q."""
    QB = 512
    m = np.zeros((128, 4 * QB), dtype=np.float32)
    for c in range(4):
        k = np.arange(128)[:, None]
        q = np.arange(QB)[None, :]
        m[:, c * QB:(c + 1) * QB] = (128 * c + k <= q).astype(np.float32)
    return m


def make_core_inputs(x, Wqkv, bqkv, core, T=T_FULL):
    """Host-side shard prep for one core."""
    b, g = divmod(core, GROUPS)
    h0 = HPC * g  # first global head of this core
    xT = np.ascontiguousarray(x[b].T).astype(np.float32)          # [C, T]

    def wq(h):
        return Wqkv[:, h * D:(h + 1) * D]

    def wk(h):
        return Wqkv[:, C + h * D: C + (h + 1) * D]

    def bq(h):
        return bqkv[h * D:(h + 1) * D]

    def bk(h):
        return bqkv[C + h * D: C + (h + 1) * D]

    z64 = np.zeros((C, D), dtype=np.float32)
    wqk = np.concatenate(
        [wq(h0), wq(h0 + 1), wk(h0), wk(h0 + 1), wq(h0 + 2), wk(h0 + 2), z64, z64],
        axis=1,
    ).astype(np.float32)                                           # [C, 512]
    zb = np.zeros(D, dtype=np.float32)
    bqk = np.stack(
        [
            np.concatenate([bq(h0), bq(h0 + 1)]),
            np.concatenate([bk(h0), bk(h0 + 1)]),
            np.concatenate([bq(h0 + 2), bk(h0 + 2)]),
            np.concatenate([zb, zb]),
        ],
        axis=1,
    ).astype(np.float32)                                           # [128, 4]
    wv = np.zeros((C, 256), dtype=np.float32)
    wv[:, : HPC * D] = Wqkv[:, 2 * C + g * HPC * D: 2 * C + (g + 1) * HPC * D]
    return {
        "xT": xT,
        "wqk": np.ascontiguousarray(wqk),
        "bqk": np.ascontiguousarray(bqk),
        "wv": wv,
        "masks": make_masks(),
        "ident": np.concatenate(
            [np.zeros((64, 64), np.float32), np.eye(64, dtype=np.float32)]
        ),
    }


_NC_CACHE = {}


def kernel(x, Wqkv, bqkv, Wout, bout):
    x = np.asarray(x, dtype=np.float32)
    Wqkv = np.asarray(Wqkv, dtype=np.float32)
    bqkv = np.asarray(bqkv, dtype=np.float32)
    Wout = np.asarray(Wout, dtype=np.float32)
    bout = np.asarray(bout, dtype=np.float32)
    T = x.shape[1]

    if T not in _NC_CACHE:
        _NC_CACHE[T] = build_nc(T)
    nc = _NC_CACHE[T]

    in_maps = []
    for core in range(N_CORES):
        b, g = divmod(core, GROUPS)
        m = make_core_inputs(x, Wqkv, bqkv, core, T)
        m["wout"] = np.ascontiguousarray(Wout[g * HPC * D:(g + 1) * HPC * D, :])
        in_maps.append(m)

    trace = bool(int(os.environ.get("KERNEL_TRACE", "0")))
    res = bass_utils.run_bass_kernel_spmd(
        nc, in_maps, core_ids=list(range(N_CORES)), trace=trace,
    )
    if trace and res.exec_time_ns is not None:
        print(f"HW exec time: {res.exec_time_ns} ns")
        if res.instructions_and_trace is not None:
            print(f"trace: {res.instructions_and_trace[1]}")

    out = np.zeros((B, T, C), dtype=np.float32)
    for b in range(B):
        for g in range(GROUPS):
            out[b] += res.results[b * GROUPS + g]["out"]
    # host bias compensation: v-bias flows through Wout as a constant row; + bout
    extra = bqkv[2 * C: 3 * C] @ Wout + bout
    out += extra[None, None, :]
    return out


# revision 25
# speedup vs baseline: 1.0126x; 1.0126x over previous
"""Trainium2 Bass kernel for causal multi-head attention (B=2, T=4096, C=768, H=12).

Sharding: 8 cores = 2 (batch) x 4 (head groups of 3). Each core computes, for its
batch element b and its 3 heads:
  - Q^T/K^T projection (transposed layout, fused q/k bias)
  - V projection (natural layout, ones column appended for free softmax sums)
  - causal flash attention in S^T = [keys, queries] orientation
  - row-sharded output projection -> partial [T, C] output
Host sums the 4 partial outputs per batch element and adds the bias terms.

All matmuls run as float32r (FP22: 1 row/cycle at N>=256), fp32 accumulate.

v8 structure (from trace analysis; HW exec: 598.6us baseline -> ~420us):
  - Q^T lives separately from K^T; K^T is split per head and zero-padded to
    K=128 contraction rows (same-tile operands and mixed-K interleave both
    slow the PE; the zero rows annihilate the other head's Q rows in the
    shared qt rhs).
  - Flash attention runs as a single global software pipeline across blocks:
    a pending queue holds emitted S-score items; exp/mask/AV for an item is
    emitted just before its psum score tile is recycled (3-buffer ring), so
    the PE streams S of items i+2..3 while ACT runs exp of item i.
  - The next block's QKV projections and this block's output projection are
    interleaved with the next block's first-head attention items, keeping exp
    work queued on ACT while the PE does projection matmuls.
  - Softmax denominators: ACT Copy (in every table set -> no table reload)
    stages the psum sums row; reciprocal_approx_fast on DVE; partition
    broadcast on GPSIMD.
"""

import os
import sys

for _p in ("/opt/trn_rl_repo", "/root/.axon_site/_ro/trn_rl_repo"):
    if os.path.isdir(_p) and _p not in sys.path:
        sys.path.insert(0, _p)

import ml_dtypes
import numpy as np

import concourse.bass as bass
import concourse.mybir as mybir
import concourse.tile as tile
from concourse import bacc, bass_utils

# Problem constants (hardcoded per harness contract)
B, T_FULL, C = 2, 4096, 768
H, D = 12, 64
N_CORES = 8
HPC = 3          # heads per core
GROUPS = 4       # head groups (cores per batch element)

F32 = mybir.dt.float32
F32R = mybir.dt.float32r


def build_nc(T=T_FULL):
    """Build the per-core Bass module. T must be a multiple of 512."""
    QB = 512                 # query block
    KC = 128                 # key chunk
    NTB = T // QB            # token blocks
    NCC = C // 128           # contraction chunks (6)
    NKC = T // KC            # key chunks total
    CS = HPC * (D + 1)       # V|ones chunk stride (195)
    CO = C                   # output channels
    VW = HPC * D             # v width (192)
    VWP = 256                # padded v proj width

    nc = bacc.Bacc(None, target_bir_lowering=False, debug=False)

    xT_d = nc.dram_tensor("xT", [C, T], F32R, kind="ExternalInput")
    wqk_d = nc.dram_tensor("wqk", [C, 4 * 128], F32R, kind="ExternalInput")
    bqk_d = nc.dram_tensor("bqk", [128, 4], F32, kind="ExternalInput")
    wv_d = nc.dram_tensor("wv", [C, VWP], F32R, kind="ExternalInput")
    wout_d = nc.dram_tensor("wout", [VW, CO], F32R, kind="ExternalInput")
    masks_d = nc.dram_tensor("masks", [128, 4 * QB], F32R, kind="ExternalInput")
    ident_d = nc.dram_tensor("ident", [128, 64], F32R, kind="ExternalInput")
    out_d = nc.dram_tensor("out", [T, CO], F32, kind="ExternalOutput")

    # per-head addressing into qt/kt tiles: block 0 = h0@p0-63|h1@p64-127,
    # block 1 = h2@p0-63
    def qbase(h):
        return 64 if h == 1 else 0

    def hoff(h):
        return T if h == 2 else 0

    with tile.TileContext(nc) as tc:
        with (
            tc.tile_pool(name="singles", bufs=1) as singles,
            tc.tile_pool(name="xt", bufs=12) as xt_pool,
            tc.tile_pool(name="e", bufs=3) as e_pool,
            tc.tile_pool(name="yt", bufs=4) as yt_pool,
            tc.tile_pool(name="nrm", bufs=2) as nrm_pool,
            tc.tile_pool(name="ostage", bufs=2) as out_pool,
            tc.tile_pool(name="ps", bufs=3, space="PSUM") as psum_s,
            tc.tile_pool(name="pz", bufs=2, space="PSUM") as psum_z,
        ):
            # Persistent SBUF tensors
            qt = singles.tile([128, 2 * T], F32R)
            # K^T per head, zero-padded to 128 contraction rows: the zero
            # half annihilates the other head's Q rows in the shared qt rhs,
            # letting every S matmul run at K=128 (matching AV's shape --
            # mixed-K interleave costs ~50ns/matmul extra in weight loads).
            kt0 = singles.tile([128, T], F32R)
            kt1 = singles.tile([128, T], F32R)
            kt2 = singles.tile([128, T], F32R)
            v1 = singles.tile([128, NKC * CS], F32R)      # V|ones, keys on partitions
            wqk_s = singles.tile([128, NCC * 512], F32R)
            wv_s = singles.tile([128, NCC * VWP], F32R)
            wout_s = singles.tile([64, HPC * CO], F32R)
            masks_s = singles.tile([128, 4 * QB], F32R)
            bqk_s = singles.tile([128, 4], F32)
            ident_s = singles.tile([128, 64], F32R)

            # ones columns of v1 (memset can't write f32r; DVE copy rounds)
            ones_c = singles.tile([128, 1], F32)
            nc.vector.memset(ones_c[:], 1.0)
            ones_dst = v1[:].rearrange("p (k h x) -> p k h x", h=HPC, x=D + 1)[
                :, :, :, D:D + 1
            ]
            nc.vector.tensor_copy(ones_dst, ones_c.to_broadcast([128, NKC, HPC, 1]))
            zero_c = singles.tile([64, 1], F32)
            nc.vector.memset(zero_c[:], 0.0)
            nc.vector.tensor_copy(kt0[64:128, :], zero_c.to_broadcast([64, T]))
            nc.vector.tensor_copy(kt1[0:64, :], zero_c.to_broadcast([64, T]))
            nc.vector.tensor_copy(kt2[64:128, :], zero_c.to_broadcast([64, T]))
            # qt block 1 upper half is never written but is read (and
            # annihilated by kt2's zero rows) by h2's K=128 S matmuls --
            # zero it so stray NaN bit patterns can't poison 0*x.
            nc.vector.tensor_copy(qt[64:128, T:2 * T], zero_c.to_broadcast([64, T]))
            nc.sync.dma_start(out=bqk_s[:], in_=bqk_d.ap())
            nc.sync.dma_start(out=ident_s[:], in_=ident_d.ap())

            def issue_xt_dma(tb):
                lst = []
                for c in range(NCC):
                    t_ = xt_pool.tile([128, QB], F32R, tag="xt", name=f"xt{tb}_{c}")
                    nc.sync.dma_start(
                        out=t_[:],
                        in_=xT_d.ap()[c * 128:(c + 1) * 128, tb * QB:(tb + 1) * QB],
                    )
                    lst.append(t_)
                return lst

            # ---- skewed cross-block pipeline ----
            # One global pending queue of attention items spans all blocks.
            # All transient psum (S scores, qk/v projections, outproj) shares
            # the 3-buffer [128,1024] "ps" ring; before each ring allocation,
            # pending items whose score tile is about to be recycled are
            # drained (exp/mask/AV emitted). This lets the next block's h0
            # attention items interleave with projections and outproj, so the
            # ACT engine always has exp work queued while the PE streams
            # projection matmuls.
            kts = [kt0, kt1, kt2]
            pending = []
            pzs = {}
            ps_ctr = [0]

            def emit_eav(j, h, m, ps, yts):
                nb = 2 * (j + 1)
                pz = pzs[(j, h)]
                e = e_pool.tile([128, 2 * QB], F32R, tag="e", name=f"e{j}_{h}_{m}")
                trims = [max(0, (2 * m + u - 4 * j) * KC) for u in range(2)]
                if trims[0] == 0 and trims[1] == 0:
                    nc.scalar.activation(
                        e[:], ps[:], mybir.ActivationFunctionType.Exp,
                        scale=0.125,
                    )
                else:
                    for u in range(2):
                        lo = u * QB + trims[u]
                        nc.scalar.activation(
                            e[:, lo:(u + 1) * QB], ps[:, lo:(u + 1) * QB],
                            mybir.ActivationFunctionType.Exp,
                            scale=0.125,
                        )
                for u in range(2):
                    cdiag = 2 * m + u - 4 * j
                    if cdiag >= 0:
                        trim = trims[u]
                        nc.vector.tensor_mul(
                            e[:, u * QB + trim:(u + 1) * QB],
                            e[:, u * QB + trim:(u + 1) * QB],
                            masks_s[:, cdiag * QB + trim:(cdiag + 1) * QB],
                        )
                for u in range(2):
                    n = 2 * m + u
                    trim = trims[u]
                    nc.tensor.matmul(
                        pz[0:D + 1, trim:QB],
                        lhsT=(v1[:, n * CS + h * (D + 1): n * CS + (h + 1) * (D + 1)]),
                        rhs=(e[:, u * QB + trim:(u + 1) * QB]),
                        start=(m == 0 and u == 0),
                        stop=(m == nb - 1 and u == 1),
                    )
                if m == nb - 1:
                    # normalize: y = z * (1/sums). ACT Copy stages the sums row
                    # from psum partition 64 to sbuf partition 0 (Copy is in
                    # every ACT table set -> no table reload); then DVE
                    # reciprocal + GPSIMD partition broadcast.
                    sums = nrm_pool.tile([1, QB], F32, tag="sums")
                    nc.scalar.activation(
                        sums[:], pz[D:D + 1, 0:QB],
                        mybir.ActivationFunctionType.Copy,
                    )
                    rc = nrm_pool.tile([1, QB], F32, tag="rc")
                    nc.vector.reciprocal_approx_fast(out=rc[:], in_=sums[:])
                    bc = nrm_pool.tile([64, QB], F32, tag="bc")
                    nc.gpsimd.partition_broadcast(bc[:], rc[:])
                    yt = yt_pool.tile([64, QB], F32R, tag="yt", name=f"yt{j}_{h}")
                    nc.vector.tensor_mul(yt[:], pz[0:D, 0:QB], bc[:])
                    yts.append(yt)
                    del pzs[(j, h)]

            def alloc_ps(name):
                # ps ring has 3 buffers: alloc #c reuses the buffer of alloc
                # #(c-3), so any pending item holding that tile must have its
                # consumers (exp) emitted before this allocation.
                c = ps_ctr[0]
                while pending and pending[0][5] <= c - 3:
                    j_, h_, m_, ps_, yts_, _ = pending.pop(0)
                    emit_eav(j_, h_, m_, ps_, yts_)
                t = psum_s.tile([128, 2 * QB], F32, tag="ps", name=name)
                ps_ctr[0] += 1
                return t

            def push(j, h, m, yts):
                if m == 0:
                    pzs[(j, h)] = psum_z.tile(
                        [128, 512], F32, tag="pz", name=f"pz{j}_{h}"
                    )
                ps = alloc_ps(f"s{j}_{h}_{m}")
                ho_ = hoff(h)
                for u in range(2):
                    n = 2 * m + u
                    trim = max(0, (n - 4 * j) * KC)
                    nc.tensor.matmul(
                        ps[:, u * QB + trim:(u + 1) * QB],
                        lhsT=(kts[h][:, n * KC:(n + 1) * KC]),
                        rhs=(qt[:, ho_ + j * QB + trim: ho_ + (j + 1) * QB]),
                        start=True,
                        stop=True,
                    )
                pending.append((j, h, m, ps, yts, ps_ctr[0] - 1))

            def flush():
                while pending:
                    j_, h_, m_, ps_, yts_, _ = pending.pop(0)
                    emit_eav(j_, h_, m_, ps_, yts_)

            def emit_qkproj_mt(tb, xt, mt):
                # M-tiles: 0 = Q(h0)|Q(h1), 1 = K(h0)|K(h1), 2 = Q(h2)|K(h2).
                # K(h2) lands on psum partitions 64-127 but must live at 0-63:
                # identity-shift via PE.
                ps = alloc_ps(f"pj{tb}_{mt}")
                for c in range(NCC):
                    nc.tensor.matmul(
                        ps[:, 0:QB],
                        lhsT=(wqk_s[:, c * 512 + mt * 128: c * 512 + (mt + 1) * 128]),
                        rhs=(xt[c][:]),
                        start=(c == 0),
                        stop=(c == NCC - 1),
                    )
                if mt == 0:
                    nc.vector.tensor_scalar(
                        out=qt[:, tb * QB:(tb + 1) * QB],
                        in0=ps[:, 0:QB],
                        scalar1=bqk_s[:, 0:1],
                        scalar2=None,
                        op0=mybir.AluOpType.add,
                    )
                elif mt == 1:
                    nc.vector.tensor_scalar(
                        out=kt0[0:64, tb * QB:(tb + 1) * QB],
                        in0=ps[0:64, 0:QB],
                        scalar1=bqk_s[0:64, 1:2],
                        scalar2=None,
                        op0=mybir.AluOpType.add,
                    )
                    nc.vector.tensor_scalar(
                        out=kt1[64:128, tb * QB:(tb + 1) * QB],
                        in0=ps[64:128, 0:QB],
                        scalar1=bqk_s[64:128, 1:2],
                        scalar2=None,
                        op0=mybir.AluOpType.add,
                    )
                else:
                    nc.vector.tensor_scalar(
                        out=qt[0:64, T + tb * QB: T + (tb + 1) * QB],
                        in0=ps[0:64, 0:QB],
                        scalar1=bqk_s[0:64, 2:3],
                        scalar2=None,
                        op0=mybir.AluOpType.add,
                    )
                    ktmp = e_pool.tile([128, 2 * QB], F32R, tag="e", name=f"ktmp{tb}")
                    nc.vector.tensor_scalar(
                        out=ktmp[64:128, 0:QB],
                        in0=ps[64:128, 0:QB],
                        scalar1=bqk_s[64:128, 2:3],
                        scalar2=None,
                        op0=mybir.AluOpType.add,
                    )
                    ps2 = alloc_ps(f"pk{tb}")
                    nc.tensor.matmul(
                        ps2[0:64, 0:QB],
                        lhsT=(ident_s[64:128, :]),
                        rhs=(ktmp[64:128, 0:QB]),
                        start=True,
                        stop=True,
                    )
                    nc.vector.tensor_copy(
                        kt2[0:64, tb * QB:(tb + 1) * QB],
                        ps2[0:64, 0:QB],
                    )

            def emit_vproj_ts(tb, xt, ts):
                pv = alloc_ps(f"pv{tb}_{ts}")
                for c in range(NCC):
                    nc.tensor.matmul(
                        pv[:, 0:VWP],
                        lhsT=(xt[c][:, ts * 128:(ts + 1) * 128]),
                        rhs=(wv_s[:, c * VWP:(c + 1) * VWP]),
                        start=(c == 0),
                        stop=(c == NCC - 1),
                    )
                kc = tb * (QB // 128) + ts
                dst = v1[:, kc * CS:(kc + 1) * CS].rearrange(
                    "p (h x) -> p h x", x=D + 1
                )[:, :, 0:D]
                src = pv[:, 0:VW].rearrange("p (h d) -> p h d", d=D)
                nc.vector.tensor_copy(dst, src)

            def emit_outproj_ts(tb, yts, ts):
                ot = out_pool.tile([128, CO], F32, tag="ot", name=f"ot{tb}_{ts}")
                for half in range(2):
                    po = alloc_ps(f"po{tb}_{ts}_{half}")
                    for h in range(HPC):
                        nc.tensor.matmul(
                            po[:, 0:384],
                            lhsT=(yts[h][:, ts * 128:(ts + 1) * 128]),
                            rhs=(wout_s[:, h * CO + half * 384: h * CO + (half + 1) * 384]),
                            start=(h == 0),
                            stop=(h == HPC - 1),
                        )
                    nc.vector.tensor_copy(
                        ot[:, half * 384:(half + 1) * 384], po[:, 0:384]
                    )
                row = tb * QB + ts * 128
                nc.sync.dma_start(out=out_d.ap()[row:row + 128, :], in_=ot[:])

            # ---- main schedule ----
            ydict = {}
            xt_cur = issue_xt_dma(0)
            for c in range(NCC):
                nc.sync.dma_start(
                    out=wqk_s[:, c * 512:(c + 1) * 512],
                    in_=wqk_d.ap()[c * 128:(c + 1) * 128, :],
                )
                nc.sync.dma_start(
                    out=wv_s[:, c * VWP:(c + 1) * VWP],
                    in_=wv_d.ap()[c * 128:(c + 1) * 128, :],
                )
            nc.sync.dma_start(out=masks_s[:], in_=masks_d.ap())
            for h_ in range(HPC):
                nc.sync.dma_start(
                    out=wout_s[:, h_ * CO:(h_ + 1) * CO],
                    in_=wout_d.ap()[h_ * D:(h_ + 1) * D, :],
                )
            for mt in range(3):
                emit_qkproj_mt(0, xt_cur, mt)
            for ts in range(QB // 128):
                emit_vproj_ts(0, xt_cur, ts)
            ydict[0] = []
            for m in range(2):
                push(0, 0, m, ydict[0])

            for tb in range(NTB):
                j = tb
                nb = 2 * (j + 1)
                yts = ydict[tb]
                xt_next = issue_xt_dma(tb + 1) if tb + 1 < NTB else None
                for m in range(nb):
                    push(j, 1, m, yts)
                for m in range(nb):
                    push(j, 2, m, yts)
                # Drain the h2 tail now: its exps + the normalize ACT Copy
                # must enter the ACT queue BEFORE the next block's exp
                # backlog, or outproj stalls ~3us per block waiting on yts.
                # No pipeline cost: these items' S matmuls are already
                # emitted, and the next work (qkproj) doesn't depend on them.
                flush()
                if tb + 1 < NTB:
                    jn = tb + 1
                    ydict[jn] = []
                    emit_qkproj_mt(jn, xt_next, 0)
                    emit_qkproj_mt(jn, xt_next, 1)
                    work = [lambda jn=jn, x=xt_next: emit_qkproj_mt(jn, x, 2)]
                    work += [
                        (lambda jn=jn, x=xt_next, t=t: emit_vproj_ts(jn, x, t))
                        for t in range(QB // 128)
                    ]
                    work += [
                        (lambda tb=tb, y=yts, t=t: emit_outproj_ts(tb, y, t))
                        for t in range(QB // 128)
                    ]
                    pushes = [
                        (lambda jn=jn, m=m: push(jn, 0, m, ydict[jn]))
                        for m in range(2 * (jn + 1))
                    ]
                    pi = wi = 0
                    while pi < len(pushes) or wi < len(work):
                        if pi < len(pushes):
                            pushes[pi]()
                            pi += 1
                        if wi < len(work):
                            work[wi]()
                            wi += 1
                    ydict.pop(tb)
                else:
                    flush()
                    for ts in range(QB // 128):
                        emit_outproj_ts(tb, yts, ts)
                    ydict.pop(tb)

    nc.compile()
    return nc


def make_masks():
    """Diagonal-block masks: masks[k, c*512 + q] = 1.0 iff 128*c + k <= q."""
    QB = 512
    m = np.zeros((128, 4 * QB), dtype=np.float32)
    for c in range(4):
        k = np.arange(128)[:, None]
        q = np.arange(QB)[None, :]
        m[:, c * QB:(c + 1) * QB] = (128 * c + k <= q).astype(np.float32)
    return m


def make_core_inputs(x, Wqkv, bqkv, core, T=T_FULL):
    """Host-side shard prep for one core."""
    b, g = divmod(core, GROUPS)
    h0 = HPC * g  # first global head of this core
    xT = np.ascontiguousarray(x[b].T).astype(np.float32)          # [C, T]

    def wq(h):
        return Wqkv[:, h * D:(h + 1) * D]

    def wk(h):
        return Wqkv[:, C + h * D: C + (h + 1) * D]

    def bq(h):
        return bqkv[h * D:(h + 1) * D]

    def bk(h):
        return bqkv[C + h * D: C + (h + 1) * D]

    z64 = np.zeros((C, D), dtype=np.float32)
    wqk = np.concatenate(
        [wq(h0), wq(h0 + 1), wk(h0), wk(h0 + 1), wq(h0 + 2), wk(h0 + 2), z64, z64],
        axis=1,
    ).astype(np.float32)                                           # [C, 512]
    zb = np.zeros(D, dtype=np.float32)
    bqk = np.stack(
        [
            np.concatenate([bq(h0), bq(h0 + 1)]),
            np.concatenate([bk(h0), bk(h0 + 1)]),
            np.concatenate([bq(h0 + 2), bk(h0 + 2)]),
            np.concatenate([zb, zb]),
        ],
        axis=1,
    ).astype(np.float32)                                           # [128, 4]
    wv = np.zeros((C, 256), dtype=np.float32)
    wv[:, : HPC * D] = Wqkv[:, 2 * C + g * HPC * D: 2 * C + (g + 1) * HPC * D]
    return {
        "xT": xT,
        "wqk": np.ascontiguousarray(wqk),
        "bqk": np.ascontiguousarray(bqk),
        "wv": wv,
        "masks": make_masks(),
        "ident": np.concatenate(
            [np.zeros((64, 64), np.float32), np.eye(64, dtype=np.float32)]
        ),
    }


_NC_CACHE = {}


def kernel(x, Wqkv, bqkv, Wout, bout):
    x = np.asarray(x, dtype=np.float32)
    Wqkv = np.asarray(Wqkv, dtype=np.float32)
    bqkv = np.asarray(bqkv, dtype=np.float32)
    Wout = np.asarray(Wout, dtype=np.float32)
    bout = np.asarray(bout, dtype=np.float32)
    T = x.shape[1]

    if T not in _NC_CACHE:
        _NC_CACHE[T] = build_nc(T)
    nc = _NC_CACHE[T]

    in_maps = []
    for core in range(N_CORES):
        b, g = divmod(core, GROUPS)
        m = make_core_inputs(x, Wqkv, bqkv, core, T)
        m["wout"] = np.ascontiguousarray(Wout[g * HPC * D:(g + 1) * HPC * D, :])
        in_maps.append(m)

    trace = bool(int(os.environ.get("KERNEL_TRACE", "0")))
    res = bass_utils.run_bass_kernel_spmd(
        nc, in_maps, core_ids=list(range(N_CORES)), trace=trace,
    )
    if trace and res.exec_time_ns is not None:
        print(f"HW exec time: {res.exec_time_ns} ns")
        if res.instructions_and_trace is not None:
            print(f"trace: {res.instructions_and_trace[1]}")

    out = np.zeros((B, T, C), dtype=np.float32)
    for b in range(B):
        for g in range(GROUPS):
            out[b] += res.results[b * GROUPS + g]["out"]
    # host bias compensation: v-bias flows through Wout as a constant row; + bout
    extra = bqkv[2 * C: 3 * C] @ Wout + bout
    out += extra[None, None, :]
    return out
